# revision 26
# baseline (speedup 1.0000x reference)
"""NTM scatter-memory kernel for 8 Trainium2 NeuronCores (Bass/Tile).

Sharding: the [8192, 4096] memory is row-sharded across 8 cores; each
core's 1024x4096 shard lives in SBUF (fp16) for all 8 steps (the final
memory is never returned, so there is no HBM traffic for it inside the
loop).

Per step:
  - controller / write-key / erase / add vectors are computed on TensorE
    from SBUF-resident weights (loaded once), with the controller vector
    replicated across all 128 output partitions (stride-0 lhsT).
  - content-addressing logits z = mem @ k and row norms are fused DVE
    scalar_tensor_tensor / ScalarE activation(accum_out) passes.
  - global softmax over 8192 slots is flash-style: AllGather of per-core
    (max, sum), local exp with global stats.
  - the rank-1 erase/add write is done in place on the SBUF shard.
  - read vector: TensorE weighted row-sum -> per-core partial read, scaled
    by the flash combine weight, AllGather -> 8 partials, combined by 8
    accumulating TensorE matmuls directly into the X update.

Activation-table discipline: two sets per step (sigmoid_and_others for
the tanh/sigmoid block at step start, natural_log_exp_and_others for
everything else; sqrt is computed as exp(0.5*ln(x))).

Self-contained: shapes hardcoded; host prep in numpy.
"""

import numpy as np

M_SLOTS = 8192
N_DIM = 4096
FVS = 64
PLEN = 64
CDIM = 256
NIN, NOUT = 512, 512
NSTEPS = 8
EPS = 1e-8

N_CORES = 8
M_LOC = M_SLOTS // N_CORES          # 1024 rows per core
RT = M_LOC // 128                   # 8 row-tiles per core
NCH = N_DIM // 512                  # 8 column chunks of 512 (psum bank)

MEM_DT = "f16"                      # memory shard dtype: "f32"|"bf16"|"f16"
K_PRE = 3                           # update tiles prestaged into AG window

_CACHE = {}


def build_nc(steps=NSTEPS, mem_dt=MEM_DT):
    import concourse.bacc as bacc
    import concourse.mybir as mybir
    import concourse.tile as tile
    from concourse.bass_isa import ReduceOp

    F32 = mybir.dt.float32
    BF16 = mybir.dt.bfloat16
    F16 = mybir.dt.float16
    MDT = {"f32": F32, "bf16": BF16, "f16": F16}[mem_dt]
    AL = mybir.AluOpType
    ACT = mybir.ActivationFunctionType
    AX = mybir.AxisListType

    try:
        import concourse.tile_utils as tile_utils
        tile_utils.max_sbuf_usage = 208 * 1024
    except Exception:
        pass

    nc = bacc.Bacc("TRN2", target_bir_lowering=False, debug=False,
                   num_devices=N_CORES)

    d_mem = nc.dram_tensor("mem", [128, RT * N_DIM], MDT, kind="ExternalInput")
    d_sqrtn0 = nc.dram_tensor("sqrtn0", [128, RT], F32, kind="ExternalInput")
    d_x0 = nc.dram_tensor("x0col", [FVS, 1], F32, kind="ExternalInput")
    d_prog = nc.dram_tensor("progpad", [128, NSTEPS], F32, kind="ExternalInput")
    d_wct = nc.dram_tensor("wct", [128, CDIM], F32, kind="ExternalInput")
    d_bc = nc.dram_tensor("bchalf", [128, 2], F32, kind="ExternalInput")
    d_wt = nc.dram_tensor("wt", [CDIM, 3 * N_DIM], BF16, kind="ExternalInput")
    # packed consts: rows 0/32/64 = bk/be/ba, rows 72..79 = kr_t
    d_krb = nc.dram_tensor("krbias", [128, N_DIM], BF16, kind="ExternalInput")
    d_oe = nc.dram_tensor("oesb", [FVS, NOUT], F32, kind="ExternalInput")
    d_ones = nc.dram_tensor("onesrow", [128, 128], BF16, kind="ExternalInput")
    d_out = nc.dram_tensor("out", [1, NOUT], F32, kind="ExternalOutput")

    RG = [list(range(N_CORES))]

    with tile.TileContext(nc) as tc:
        with (
            tc.tile_pool(name="pmem", bufs=1) as pmem,
            tc.tile_pool(name="pconst", bufs=1) as pconst,
            tc.tile_pool(name="pstate", bufs=2) as pstate,
            tc.tile_pool(name="pvb", bufs=3) as pvb,
            tc.tile_pool(name="pscr", bufs=5) as pscr,
            tc.tile_pool(name="psm", bufs=3) as psm,
            tc.tile_pool(name="prow", bufs=1) as prow,
            tc.tile_pool(name="prp", bufs=2) as prp,
            tc.tile_pool(name="pkr", bufs=1) as pkr,
            tc.tile_pool(name="prc", bufs=1) as prc,
            tc.tile_pool(name="pps", bufs=2, space="PSUM") as pps,
            tc.tile_pool(name="ppsb", bufs=2, space="PSUM") as ppsb,
            tc.tile_pool(name="ppsc", bufs=1, space="PSUM") as ppsc,
            tc.tile_pool(name="pdram", bufs=4, space="DRAM") as pdram,
        ):
            # ---- persistent state + resident weights ----
            mem = pmem.tile([128, RT * N_DIM], MDT, tag="mem")
            nc.sync.dma_start(mem[:], d_mem[:])
            sqrtn = pstate.tile([128, RT], F32, tag="sqrtn")
            nc.sync.dma_start(sqrtn[:], d_sqrtn0[:])
            x_col = pstate.tile([FVS, 1], F32, tag="xcol")
            nc.sync.dma_start(x_col[:], d_x0[:])

            prog = pconst.tile([128, NSTEPS], F32, tag="prog")
            nc.sync.dma_start(prog[:], d_prog[:])
            wct = pconst.tile([128, CDIM], F32, tag="wct")
            nc.sync.dma_start(wct[:], d_wct[:])
            bchalf = pconst.tile([128, 2], F32, tag="bchalf")
            nc.sync.dma_start(bchalf[:], d_bc[:])
            oesb = pconst.tile([FVS, NOUT], F32, tag="oesb")
            nc.sync.dma_start(oesb[:], d_oe[:])
            onesb = pconst.tile([128, 128], BF16, tag="onesb")
            nc.sync.dma_start(onesb[:], d_ones[:])
            wtA = pconst.tile([128, 3 * N_DIM], BF16, tag="wtA")
            nc.sync.dma_start(wtA[:], d_wt[0:128, :])
            wtB = pconst.tile([128, 3 * N_DIM], BF16, tag="wtB")
            nc.sync.dma_start(wtB[:], d_wt[128:256, :])
            krb = pconst.tile([128, N_DIM], BF16, tag="krb")
            nc.sync.dma_start(krb[:], d_krb[:])
            negone = pconst.tile([128, 1], F32, tag="negone")
            nc.vector.memset(negone[:], -1.0)

            def msl(rt):
                return slice(rt * N_DIM, (rt + 1) * N_DIM)

            # small-op helpers -------------------------------------------
            def neg_of(ap, tag):
                t = psm.tile([ap.shape[0], 1], F32, tag=tag)
                nc.vector.tensor_scalar(t[:], ap, -1.0, None, AL.mult)
                return t

            def sqrt_lnexp(out, in_, tagp):
                """out = sqrt(in_) = exp(0.5*ln(in_)); stays in lnexp set."""
                ln = psm.tile([in_.shape[0], in_.shape[1]], F32, tag=tagp + "ln")
                nc.scalar.activation(ln[:], in_, ACT.Ln)
                nc.scalar.activation(out, ln[:], ACT.Exp, scale=0.5)

            x_ps_prev = None  # PSUM [FVS,1] holding pre-tanh X (cross-step)

            for t in range(steps):
                # ---------- X tanh (prev step) + controller, SIG set ----
                if x_ps_prev is not None:
                    x_new = pstate.tile([FVS, 1], F32, tag="xcol")
                    nc.scalar.activation(x_new[:], x_ps_prev[:], ACT.Tanh)
                    x_col = x_new
                cat = psm.tile([128, 1], F32, tag="cat")
                nc.vector.tensor_copy(cat[FVS:128, :], prog[FVS:128, t:t + 1])
                nc.vector.tensor_copy(cat[0:FVS, :], x_col[:])
                c_ps = ppsc.tile([128, 2], F32, tag="mini")
                nc.tensor.matmul(c_ps[:, 0:1], wct[:, 0:128], cat[:],
                                 start=True, stop=True)
                nc.tensor.matmul(c_ps[:, 1:2], wct[:, 128:256], cat[:],
                                 start=True, stop=True)
                # sigmoid(y) = 0.5 + 0.5*tanh(0.5*y); bchalf = 0.5*bc
                c_th = psm.tile([128, 2], F32, tag="c_th")
                for h in range(2):
                    nc.scalar.activation(c_th[:, h:h + 1], c_ps[:, h:h + 1],
                                         ACT.Tanh, bias=bchalf[:, h:h + 1],
                                         scale=0.5)
                c_sb = psm.tile([128, 2], BF16, tag="c_sb")
                nc.vector.tensor_scalar(c_sb[:], c_th[:], 0.5, 0.5,
                                        AL.mult, AL.add)

                # ---------- k / e / a fused with broadcast ----------
                c0b = c_sb[:, 0:1].broadcast_to([128, 128])
                c1b = c_sb[:, 1:2].broadcast_to([128, 128])
                kea = []
                for m in range(3):
                    vb = pvb.tile([128, N_DIM], BF16, tag="vb")
                    for ch in range(NCH):
                        cbase = m * N_DIM + ch * 512
                        bc_ps = pps.tile([128, 512], F32, tag="bc_ps")
                        nc.tensor.matmul(bc_ps[:], c0b,
                                         wtA[:, cbase:cbase + 512],
                                         start=True, stop=False)
                        nc.tensor.matmul(bc_ps[:], c1b,
                                         wtB[:, cbase:cbase + 512],
                                         start=False, stop=False)
                        nc.tensor.matmul(bc_ps[:],
                                         onesb[32 * m:32 * m + 1, :],
                                         krb[32 * m:32 * m + 1,
                                             ch * 512:(ch + 1) * 512],
                                         start=False, stop=True)
                        sc = 0.5 if m == 1 else 1.0
                        nc.scalar.activation(vb[:, ch * 512:(ch + 1) * 512],
                                             bc_ps[:], ACT.Tanh, scale=sc)
                    kea.append(vb)
                k_b, e_b, a_b = kea
                # e = 0.5 + 0.5*tanh(0.5*y)  (tanh already applied above)
                nc.vector.tensor_scalar(e_b[:], e_b[:], 0.5, 0.5,
                                        AL.mult, AL.add)

                # ---------- ||k||^2 (every lane ends up with the value) ----
                kk2 = psm.tile([128, 1], F32, tag="kk2")
                dumb = psm.tile([128, 1], F32, tag="dumb")
                nc.vector.scalar_tensor_tensor(
                    dumb[:].broadcast_to([128, N_DIM]), k_b[:], 1.0, k_b[:],
                    AL.mult, AL.mult, accum_out=kk2[:])

                # ---------- z_w = mem @ k : TT product (2x) + ACT accum ----
                zw = psm.tile([128, RT], F32, tag="zw")
                for rt in range(RT):
                    scr = pscr.tile([128, N_DIM], MDT, tag="scr")
                    nc.vector.tensor_tensor(scr[:], mem[:, msl(rt)], k_b[:],
                                            AL.mult)
                    nc.scalar.activation(scr[:], scr[:], ACT.Copy,
                                         accum_out=zw[:, rt:rt + 1])

                # ---------- write logits + local stats (LNEXP set) ------
                # den = sqrtn*sqrt(kk2) + EPS ; sqrt via exp(0.5 ln x)
                kk = psm.tile([128, 1], F32, tag="kk")
                sqrt_lnexp(kk[:], kk2[:], "dw")
                den = psm.tile([128, RT], F32, tag="den")
                nc.vector.tensor_scalar(den[:], sqrtn[:], kk[:], EPS,
                                        AL.mult, AL.add)
                rec = psm.tile([128, RT], F32, tag="rec")
                nc.vector.reciprocal(rec[:], den[:])
                li_w = psm.tile([128, RT], F32, tag="li_w")
                nc.vector.tensor_tensor(li_w[:], zw[:], rec[:], AL.mult)
                # cosine logits are bounded by ~1, so a fixed reference
                # exp(li - 1) replaces the flash max: only the SUM needs
                # to cross cores.
                uw = psm.tile([128, RT], F32, tag="uw")
                nc.scalar.activation(uw[:], li_w[:], ACT.Exp,
                                     bias=negone[:])
                rsum_w = psm.tile([128, 1], F32, tag="rsum_w")
                nc.vector.tensor_reduce(rsum_w[:], uw[:], AX.X, AL.add)
                lsum_w = psm.tile([128, 1], F32, tag="lsum_w")
                nc.gpsimd.partition_all_reduce(lsum_w[:], rsum_w[:], 128,
                                               ReduceOp.add)

                # ---------- AllGather write stats (one scalar) ----------
                pay_a = prow.tile([1, 1], F32, tag="pay_a")
                nc.vector.tensor_copy(pay_a[0:1, 0:1], lsum_w[0:1, :])
                ag_a_in = pdram.tile([1, 1], F32, tag="ag_a_in")
                ag_a_out = pdram.tile([N_CORES, 1], F32, tag="ag_a_out")
                nc.sync.dma_start(ag_a_in[:], pay_a[:])
                nc.gpsimd.collective_compute(
                    "AllGather", AL.bypass, replica_groups=RG,
                    ins=[ag_a_in.opt()], outs=[ag_a_out.opt()])

                # ---- w-independent work emitted into the AG window ----
                krrow = pkr.tile([1, N_DIM], BF16, tag="krrow")
                nc.sync.dma_start(krrow[:], krb[72 + t:73 + t, :])
                kr_b = pvb.tile([128, N_DIM], BF16, tag="vb")
                for ch in range(NCH):
                    kr_ps = pps.tile([128, 512], F32, tag="bc_ps")
                    nc.tensor.matmul(kr_ps[:], onesb[0:1, :],
                                     krrow[0:1, ch * 512:(ch + 1) * 512],
                                     start=True, stop=True)
                    nc.vector.tensor_copy(kr_b[:, ch * 512:(ch + 1) * 512],
                                          kr_ps[:])

                def upd_p12(rt):
                    s1 = pscr.tile([128, N_DIM], MDT, tag="scr")
                    nc.vector.tensor_tensor(s1[:], mem[:, msl(rt)], e_b[:],
                                            AL.mult)
                    nc.vector.tensor_tensor(s1[:], a_b[:], s1[:], AL.subtract)
                    return s1

                pre_s1 = {rt: upd_p12(rt) for rt in range(K_PRE)}

                stw = prow.tile([1, N_CORES], F32, tag="stw")
                nc.sync.dma_start(stw[:], ag_a_out[:].rearrange("c s -> s c"))
                gsum1 = prow.tile([1, 1], F32, tag="gsum1")
                nc.vector.tensor_reduce(gsum1[:], stw[:], AX.X, AL.add)
                giv1 = prow.tile([1, 1], F32, tag="giv1")
                nc.vector.reciprocal(giv1[:], gsum1[:])
                ginv = psm.tile([128, 1], F32, tag="ginv")
                nc.gpsimd.partition_broadcast(ginv[:], giv1[:])
                w_col = psm.tile([128, RT], F32, tag="w_col")
                nc.vector.tensor_scalar(w_col[:], uw[:], ginv[:], None,
                                        AL.mult)

                # ---------- update + z_r + norms, tile by tile ----------
                def upd_p3(rt, s1):
                    # mem += w*s1 as ts(4x) + tt(2x); s1 scaled in place
                    nc.vector.tensor_scalar(s1[:], s1[:],
                                            w_col[:, rt:rt + 1], None,
                                            AL.mult)
                    nc.vector.tensor_tensor(mem[:, msl(rt)], mem[:, msl(rt)],
                                            s1[:], AL.add)

                zr = psm.tile([128, RT], F32, tag="zr")
                npc = psm.tile([128, RT], F32, tag="npc")
                for rt in range(RT):
                    s1 = pre_s1[rt] if rt in pre_s1 else upd_p12(rt)
                    upd_p3(rt, s1)
                    scr2 = pscr.tile([128, N_DIM], MDT, tag="scr")
                    nc.vector.tensor_tensor(scr2[:], mem[:, msl(rt)],
                                            kr_b[:], AL.mult)
                    nc.scalar.activation(scr2[:], scr2[:], ACT.Copy,
                                         accum_out=zr[:, rt:rt + 1])
                    nc.scalar.activation(scr2[:], mem[:, msl(rt)], ACT.Square,
                                         accum_out=npc[:, rt:rt + 1])

                # ---------- read logits + local stats ----------
                # ||kr|| == 1 (normalized on host): den_r = sqrt(npc) + EPS
                sqrtn_new = pstate.tile([128, RT], F32, tag="sqrtn")
                sqrt_lnexp(sqrtn_new[:], npc[:], "dr")
                sqrtn = sqrtn_new
                den_r = psm.tile([128, RT], F32, tag="den_r")
                nc.vector.tensor_scalar(den_r[:], sqrtn[:], EPS, None, AL.add)
                rec_r = psm.tile([128, RT], F32, tag="rec_r")
                nc.vector.reciprocal(rec_r[:], den_r[:])
                li_r = psm.tile([128, RT], F32, tag="li_r")
                nc.vector.tensor_tensor(li_r[:], zr[:], rec_r[:], AL.mult)
                u_col = psm.tile([128, RT], F32, tag="u_col")
                nc.scalar.activation(u_col[:], li_r[:], ACT.Exp,
                                     bias=negone[:])
                rsum_r = psm.tile([128, 1], F32, tag="rsum_r")
                nc.vector.tensor_reduce(rsum_r[:], u_col[:], AX.X, AL.add)
                lsum_r = psm.tile([128, 1], F32, tag="lsum_r")
                nc.gpsimd.partition_all_reduce(lsum_r[:], rsum_r[:], 128,
                                               ReduceOp.add)

                # u_col (f32) is the local unnormalized read weight; compute
                # partial read rp = sum_r u_r * mem'[r, :] on TensorE.
                ucast = psm.tile([128, RT], MDT, tag="ucast")
                if mem_dt != "f32":
                    nc.vector.tensor_copy(ucast[:], u_col[:])
                    u_lhs = ucast
                else:
                    u_lhs = u_col
                rp_pss = []
                for cc in range(8):
                    rp_ps = ppsb.tile([1, 512], F32, tag="rp_ps")
                    base = cc * 512
                    for rt in range(RT):
                        nc.tensor.matmul(
                            rp_ps[:],
                            u_lhs[:, rt:rt + 1],
                            mem[:, rt * N_DIM + base:rt * N_DIM + base + 512],
                            start=(rt == 0), stop=(rt == RT - 1))
                    rp_pss.append(rp_ps)

                # ONE AllGather carries the unscaled partial read AND the
                # local read-softmax stats; combine weights are computed
                # locally afterwards and folded into the received R-blocks.
                ag_rd_in = pdram.tile([1, N_DIM + 1], F32, tag="ag_rd_in")
                ag_rd_out = pdram.tile([N_CORES, N_DIM + 1], F32,
                                       tag="ag_rd_out")
                for cc in range(8):
                    rp_sb = prp.tile([1, 512], F32, tag="rp_sb")
                    nc.vector.tensor_copy(rp_sb[:], rp_pss[cc][:])
                    nc.sync.dma_start(ag_rd_in[0:1, cc * 512:(cc + 1) * 512],
                                      rp_sb[:])
                pay_r = prow.tile([1, 1], F32, tag="pay_a")
                nc.vector.tensor_copy(pay_r[0:1, 0:1], lsum_r[0:1, :])
                nc.sync.dma_start(ag_rd_in[0:1, N_DIM:N_DIM + 1], pay_r[:])
                nc.gpsimd.collective_compute(
                    "AllGather", AL.bypass, replica_groups=RG,
                    ins=[ag_rd_in.opt()], outs=[ag_rd_out.opt()])

                # read = (sum_c rp_c) / (sum_c rawsum_c); the division is
                # folded into the X-update matmul input.
                strsum = prow.tile([1, N_CORES], F32, tag="strsum")
                nc.sync.dma_start(
                    strsum[:],
                    ag_rd_out[:, N_DIM:N_DIM + 1].rearrange("c s -> s c"))
                dsum = prow.tile([1, 1], F32, tag="dsum")
                nc.vector.tensor_reduce(dsum[:], strsum[:], AX.X, AL.add)
                dinv = prow.tile([1, 1], F32, tag="dinv1")
                nc.vector.reciprocal(dinv[:], dsum[:])
                dinv_b = psm.tile([FVS, 1], F32, tag="dinv_b")
                nc.gpsimd.partition_broadcast(dinv_b[:], dinv[:])

                # ---------- executioner: X <- tanh((X/d) @ sum_c R^c) ------
                # rcol8[i, c*FVS + j] = R^c[i, j]
                rcol8 = prc.tile([FVS, N_CORES * FVS], F32, tag="rcol8")
                nc.sync.dma_start(
                    rcol8[:].rearrange("i (c j) -> i c j", c=N_CORES),
                    ag_rd_out[:, 0:N_DIM].rearrange("c (i j) -> i c j",
                                                    i=FVS))
                xs_col = psm.tile([FVS, 1], F32, tag="xs_col")
                nc.vector.tensor_scalar(xs_col[:], x_col[:], dinv_b[:], None,
                                        AL.mult)
                x_ps = ppsc.tile([FVS, 1], F32, tag="mini")
                for c in range(N_CORES):
                    nc.tensor.matmul(x_ps[:],
                                     rcol8[:, c * FVS:(c + 1) * FVS],
                                     xs_col[:],
                                     start=(c == 0), stop=(c == N_CORES - 1))
                x_ps_prev = x_ps

            # ---------- output: Xf @ output_embedding ----------
            x_fin = pstate.tile([FVS, 1], F32, tag="xcol")
            nc.scalar.activation(x_fin[:], x_ps_prev[:], ACT.Tanh)
            o_ps = ppsc.tile([1, NOUT], F32, tag="mini")
            nc.tensor.matmul(o_ps[:], x_fin[:], oesb[:], start=True, stop=True)
            o_sb = prow.tile([1, NOUT], F32, tag="o_sb")
            nc.vector.tensor_copy(o_sb[:], o_ps[:])
            nc.sync.dma_start(d_out[:], o_sb[:])

    nc.compile()
    return nc


def host_prep(inputs, mem_dt=MEM_DT):
    import ml_dtypes
    bf16 = ml_dtypes.bfloat16
    f32 = np.float32

    x = np.asarray(inputs["x"], f32)
    program = np.asarray(inputs["program"], f32)
    memory0 = np.asarray(inputs["memory0"], f32)
    ie = np.asarray(inputs["input_embedding"], f32)
    oe = np.asarray(inputs["output_embedding"], f32)
    Wc = np.asarray(inputs["Wc"], f32)
    bc = np.asarray(inputs["bc"], f32)
    Wk = np.asarray(inputs["Wk"], f32)
    bk = np.asarray(inputs["bk"], f32)
    We = np.asarray(inputs["We"], f32)
    be = np.asarray(inputs["be"], f32)
    Wa = np.asarray(inputs["Wa"], f32)
    ba = np.asarray(inputs["ba"], f32)
    Wrk = np.asarray(inputs["Wrk"], f32)
    brk = np.asarray(inputs["brk"], f32)

    x0col = (x @ ie).astype(f32).reshape(FVS, 1)

    progpad = np.zeros((128, NSTEPS), f32)
    progpad[FVS:128, :] = program[0].T          # rows 64:128 = prog_t

    wct = np.ascontiguousarray(Wc.T)            # [128, 256]
    # bchalf[p,h] = 0.5*bc[h*128+p]  (for sigmoid-via-tanh)
    bchalf = np.ascontiguousarray(0.5 * bc.reshape(2, 128).T)

    wt = np.concatenate([Wk.T, We.T, Wa.T], axis=1).astype(bf16)  # [256,12288]

    kr = np.tanh(program[0] @ Wrk.T + brk)      # [8, 4096]
    kr = kr / np.linalg.norm(kr, axis=1, keepdims=True)
    krbias = np.zeros((128, N_DIM), np.float32)
    krbias[0] = bk
    krbias[32] = be
    krbias[64] = ba
    krbias[72:72 + NSTEPS] = kr
    krbias = krbias.astype(bf16)

    onesrow = np.ones((128, 128), bf16)

    mdt = {"f32": f32, "bf16": bf16, "f16": np.float16}[mem_dt]
    common = {
        "x0col": x0col, "progpad": progpad, "wct": wct, "bchalf": bchalf,
        "wt": wt, "krbias": krbias,
        "oesb": np.ascontiguousarray(oe), "onesrow": onesrow,
    }
    in_maps = []
    for r in range(N_CORES):
        shard = memory0[r * M_LOC:(r + 1) * M_LOC, :]
        n = np.sqrt((shard.astype(np.float64) ** 2).sum(1)).astype(f32)
        sqrtn0 = np.ascontiguousarray(n.reshape(RT, 128).T)  # [p, t]
        m = dict(common)
        m["mem"] = np.ascontiguousarray(
            shard.reshape(RT, 128, N_DIM).transpose(1, 0, 2)
            .reshape(128, RT * N_DIM).astype(mdt))
        m["sqrtn0"] = sqrtn0
        in_maps.append(m)
    return in_maps


def kernel(**inputs):
    from concourse.bass_utils import run_bass_kernel_spmd
    key = ("nc", NSTEPS, MEM_DT)
    if key not in _CACHE:
        _CACHE[key] = build_nc(NSTEPS, MEM_DT)
    nc = _CACHE[key]
    in_maps = host_prep(inputs, MEM_DT)
    res = run_bass_kernel_spmd(nc, in_maps, core_ids=list(range(N_CORES)))
    return np.asarray(res.results[0]["out"], np.float32)


# revision 47
# speedup vs baseline: 4.3091x; 4.3091x over previous
"""NTM scatter-memory kernel for 8 Trainium2 NeuronCores (Bass/Tile).

Sharding: the [8192, 4096] memory is row-sharded across 8 cores; each
core's 1024x4096 shard lives in SBUF (fp16) for all 8 steps (the final
memory is never returned, so there is no HBM traffic for it inside the
loop).

Per step:
  - controller / write-key / erase / add vectors are computed on TensorE
    from SBUF-resident weights (loaded once), with the controller vector
    replicated across all 128 output partitions (stride-0 lhsT).
  - content-addressing logits z = mem @ k and row norms are fused DVE
    scalar_tensor_tensor / ScalarE activation(accum_out) passes.
  - global softmax over 8192 slots is flash-style: AllGather of per-core
    (max, sum), local exp with global stats.
  - the rank-1 erase/add write is done in place on the SBUF shard.
  - read vector: TensorE weighted row-sum -> per-core partial read, scaled
    by the flash combine weight, AllGather -> 8 partials, combined by 8
    accumulating TensorE matmuls directly into the X update.

Activation-table discipline: two sets per step (sigmoid_and_others for
the tanh/sigmoid block at step start, natural_log_exp_and_others for
everything else; sqrt is computed as exp(0.5*ln(x))).

Self-contained: shapes hardcoded; host prep in numpy.
"""

import numpy as np

M_SLOTS = 8192
N_DIM = 4096
FVS = 64
PLEN = 64
CDIM = 256
NIN, NOUT = 512, 512
NSTEPS = 8
EPS = 1e-8

N_CORES = 8
M_LOC = M_SLOTS // N_CORES          # 1024 rows per core
RT = M_LOC // 128                   # 8 row-tiles per core
NCH = N_DIM // 512                  # 8 column chunks of 512 (psum bank)

MEM_DT = "f16"                      # memory shard dtype: "f32"|"bf16"|"f16"
K_PRE = 3                           # update tiles prestaged into AG window
NSPLIT = 6                          # update tiles before early read-partials

_CACHE = {}


def build_nc(steps=NSTEPS, mem_dt=MEM_DT):
    import concourse.bacc as bacc
    import concourse.mybir as mybir
    import concourse.tile as tile
    from concourse.bass_isa import ReduceOp

    F32 = mybir.dt.float32
    BF16 = mybir.dt.bfloat16
    F16 = mybir.dt.float16
    MDT = {"f32": F32, "bf16": BF16, "f16": F16}[mem_dt]
    AL = mybir.AluOpType
    ACT = mybir.ActivationFunctionType
    AX = mybir.AxisListType

    try:
        import concourse.tile_utils as tile_utils
        tile_utils.max_sbuf_usage = 208 * 1024
    except Exception:
        pass

    nc = bacc.Bacc("TRN2", target_bir_lowering=False, debug=False,
                   num_devices=N_CORES)

    d_mem = nc.dram_tensor("mem", [128, RT * N_DIM], MDT, kind="ExternalInput")
    d_sqrtn0 = nc.dram_tensor("sqrtn0", [128, RT], F32, kind="ExternalInput")
    d_x0 = nc.dram_tensor("x0col", [FVS, 1], F32, kind="ExternalInput")
    d_prog = nc.dram_tensor("progpad", [128, NSTEPS], F32, kind="ExternalInput")
    d_wct = nc.dram_tensor("wct", [128, CDIM], F32, kind="ExternalInput")
    d_bc = nc.dram_tensor("bchalf", [128, 2], F32, kind="ExternalInput")
    d_wt = nc.dram_tensor("wt", [CDIM, 3 * N_DIM], BF16, kind="ExternalInput")
    # packed consts: rows 0/32/64 = bk/be/ba, rows 72..79 = kr_t
    d_krb = nc.dram_tensor("krbias", [128, N_DIM], BF16, kind="ExternalInput")
    d_oe = nc.dram_tensor("oesb", [FVS, NOUT], F32, kind="ExternalInput")
    d_ones = nc.dram_tensor("onesrow", [128, 128], BF16, kind="ExternalInput")
    d_out = nc.dram_tensor("out", [1, NOUT], F32, kind="ExternalOutput")

    RG = [list(range(N_CORES))]

    with tile.TileContext(nc) as tc:
        with (
            tc.tile_pool(name="pmem", bufs=1) as pmem,
            tc.tile_pool(name="pconst", bufs=1) as pconst,
            tc.tile_pool(name="pstate", bufs=2) as pstate,
            tc.tile_pool(name="pvb", bufs=3) as pvb,
            tc.tile_pool(name="pscr", bufs=5) as pscr,
            tc.tile_pool(name="psm", bufs=3) as psm,
            tc.tile_pool(name="prow", bufs=1) as prow,
            tc.tile_pool(name="prp", bufs=2) as prp,
            tc.tile_pool(name="pkr", bufs=1) as pkr,
            tc.tile_pool(name="prc", bufs=1) as prc,
            tc.tile_pool(name="pps", bufs=3, space="PSUM") as pps,
            tc.tile_pool(name="ppsb", bufs=3, space="PSUM") as ppsb,
            tc.tile_pool(name="ppsc", bufs=1, space="PSUM") as ppsc,
            tc.tile_pool(name="pdram", bufs=4, space="DRAM") as pdram,
        ):
            # ---- persistent state + resident weights ----
            mem = pmem.tile([128, RT * N_DIM], MDT, tag="mem")
            nc.sync.dma_start(mem[:], d_mem[:])
            sqrtn = pstate.tile([128, RT], F32, tag="sqrtn")
            nc.sync.dma_start(sqrtn[:], d_sqrtn0[:])
            x_col = pstate.tile([FVS, 1], F32, tag="xcol")
            nc.sync.dma_start(x_col[:], d_x0[:])

            prog = pconst.tile([128, NSTEPS], F32, tag="prog")
            nc.sync.dma_start(prog[:], d_prog[:])
            wct = pconst.tile([128, CDIM], F32, tag="wct")
            nc.sync.dma_start(wct[:], d_wct[:])
            bchalf = pconst.tile([128, 2], F32, tag="bchalf")
            nc.sync.dma_start(bchalf[:], d_bc[:])
            oesb = pconst.tile([FVS, NOUT], F32, tag="oesb")
            nc.sync.dma_start(oesb[:], d_oe[:])
            onesb = pconst.tile([128, 128], BF16, tag="onesb")
            nc.sync.dma_start(onesb[:], d_ones[:])
            wtA = pconst.tile([128, 3 * N_DIM], BF16, tag="wtA")
            nc.sync.dma_start(wtA[:], d_wt[0:128, :])
            wtB = pconst.tile([128, 3 * N_DIM], BF16, tag="wtB")
            nc.sync.dma_start(wtB[:], d_wt[128:256, :])
            krb = pconst.tile([128, N_DIM], BF16, tag="krb")
            nc.sync.dma_start(krb[:], d_krb[:])
            negone = pconst.tile([128, 1], F32, tag="negone")
            nc.vector.memset(negone[:], -1.0)

            def msl(rt):
                return slice(rt * N_DIM, (rt + 1) * N_DIM)

            # small-op helpers -------------------------------------------
            def neg_of(ap, tag):
                t = psm.tile([ap.shape[0], 1], F32, tag=tag)
                nc.vector.tensor_scalar(t[:], ap, -1.0, None, AL.mult)
                return t

            def sqrt_lnexp(out, in_, tagp):
                """out = sqrt(in_) = exp(0.5*ln(in_)); stays in lnexp set."""
                ln = psm.tile([in_.shape[0], in_.shape[1]], F32, tag=tagp + "ln")
                nc.scalar.activation(ln[:], in_, ACT.Ln)
                nc.scalar.activation(out, ln[:], ACT.Exp, scale=0.5)

            x_ps_prev = None  # PSUM [FVS,1] holding pre-tanh X (cross-step)

            for t in range(steps):
                # ---------- X tanh (prev step) + controller, SIG set ----
                if x_ps_prev is not None:
                    x_new = pstate.tile([FVS, 1], F32, tag="xcol")
                    nc.scalar.activation(x_new[:], x_ps_prev[:], ACT.Tanh)
                    x_col = x_new
                cat = psm.tile([128, 1], F32, tag="cat")
                nc.vector.tensor_copy(cat[FVS:128, :], prog[FVS:128, t:t + 1])
                nc.vector.tensor_copy(cat[0:FVS, :], x_col[:])
                c_ps = ppsc.tile([128, 2], F32, tag="mini")
                nc.tensor.matmul(c_ps[:, 0:1], wct[:, 0:128], cat[:],
                                 start=True, stop=True)
                nc.tensor.matmul(c_ps[:, 1:2], wct[:, 128:256], cat[:],
                                 start=True, stop=True)
                # sigmoid(y) = 0.5 + 0.5*tanh(0.5*y); bchalf = 0.5*bc
                c_th = psm.tile([128, 2], F32, tag="c_th")
                for h in range(2):
                    nc.scalar.activation(c_th[:, h:h + 1], c_ps[:, h:h + 1],
                                         ACT.Tanh, bias=bchalf[:, h:h + 1],
                                         scale=0.5)
                c_sb = psm.tile([128, 2], BF16, tag="c_sb")
                nc.vector.tensor_scalar(c_sb[:], c_th[:], 0.5, 0.5,
                                        AL.mult, AL.add)

                # ---------- k / e / a fused with broadcast ----------
                c0b = c_sb[:, 0:1].broadcast_to([128, 128])
                c1b = c_sb[:, 1:2].broadcast_to([128, 128])
                kea = []
                for m in range(3):
                    vb = pvb.tile([128, N_DIM], BF16, tag="vb")
                    for ch in range(NCH):
                        cbase = m * N_DIM + ch * 512
                        bc_ps = pps.tile([128, 512], F32, tag="bc_ps")
                        nc.tensor.matmul(bc_ps[:], c0b,
                                         wtA[:, cbase:cbase + 512],
                                         start=True, stop=False)
                        nc.tensor.matmul(bc_ps[:], c1b,
                                         wtB[:, cbase:cbase + 512],
                                         start=False, stop=False)
                        nc.tensor.matmul(bc_ps[:],
                                         onesb[32 * m:32 * m + 1, :],
                                         krb[32 * m:32 * m + 1,
                                             ch * 512:(ch + 1) * 512],
                                         start=False, stop=True)
                        sc = 0.5 if m == 1 else 1.0
                        nc.scalar.activation(vb[:, ch * 512:(ch + 1) * 512],
                                             bc_ps[:], ACT.Tanh, scale=sc)
                    kea.append(vb)
                k_b, e_b, a_b = kea
                # e = 0.5 + 0.5*tanh(0.5*y)  (tanh already applied above)
                nc.vector.tensor_scalar(e_b[:], e_b[:], 0.5, 0.5,
                                        AL.mult, AL.add)

                # ---------- ||k||^2 (every lane ends up with the value) ----
                kk2 = psm.tile([128, 1], F32, tag="kk2")
                dumb = psm.tile([128, 1], F32, tag="dumb")
                nc.vector.scalar_tensor_tensor(
                    dumb[:].broadcast_to([128, N_DIM]), k_b[:], 1.0, k_b[:],
                    AL.mult, AL.mult, accum_out=kk2[:])

                # ---------- z_w = mem @ k : TT product (2x) + ACT accum ----
                zw = psm.tile([128, RT], F32, tag="zw")
                for rt in range(RT):
                    scr = pscr.tile([128, N_DIM], MDT, tag="scr")
                    nc.vector.tensor_tensor(scr[:], mem[:, msl(rt)], k_b[:],
                                            AL.mult)
                    nc.scalar.activation(scr[:], scr[:], ACT.Copy,
                                         accum_out=zw[:, rt:rt + 1])

                # ---------- write logits + local stats (LNEXP set) ------
                # den = sqrtn*sqrt(kk2) + EPS ; sqrt via exp(0.5 ln x)
                kk = psm.tile([128, 1], F32, tag="kk")
                sqrt_lnexp(kk[:], kk2[:], "dw")
                den = psm.tile([128, RT], F32, tag="den")
                nc.vector.tensor_scalar(den[:], sqrtn[:], kk[:], EPS,
                                        AL.mult, AL.add)
                rec = psm.tile([128, RT], F32, tag="rec")
                nc.vector.reciprocal(rec[:], den[:])
                li_w = psm.tile([128, RT], F32, tag="li_w")
                nc.vector.tensor_tensor(li_w[:], zw[:], rec[:], AL.mult)
                # cosine logits are bounded by ~1, so a fixed reference
                # exp(li - 1) replaces the flash max: only the SUM needs
                # to cross cores.
                uw = psm.tile([128, RT], F32, tag="uw")
                nc.scalar.activation(uw[:], li_w[:], ACT.Exp,
                                     bias=negone[:])
                rsum_w = psm.tile([128, 1], F32, tag="rsum_w")
                nc.vector.tensor_reduce(rsum_w[:], uw[:], AX.X, AL.add)
                lsum_w = psm.tile([128, 1], F32, tag="lsum_w")
                nc.gpsimd.partition_all_reduce(lsum_w[:], rsum_w[:], 128,
                                               ReduceOp.add)

                # ---------- AllGather write stats (one scalar) ----------
                pay_a = prow.tile([1, 1], F32, tag="pay_a")
                nc.vector.tensor_copy(pay_a[0:1, 0:1], lsum_w[0:1, :])
                ag_a_in = pdram.tile([1, 1], F32, tag="ag_a_in")
                ag_a_out = pdram.tile([N_CORES, 1], F32, tag="ag_a_out")
                nc.sync.dma_start(ag_a_in[:], pay_a[:])
                nc.gpsimd.collective_compute(
                    "AllGather", AL.bypass, replica_groups=RG,
                    ins=[ag_a_in.opt()], outs=[ag_a_out.opt()])

                # ---- w-independent work emitted into the AG window ----
                krrow = pkr.tile([1, N_DIM], BF16, tag="krrow")
                nc.sync.dma_start(krrow[:], krb[72 + t:73 + t, :])
                kr_b = pvb.tile([128, N_DIM], BF16, tag="vb")
                for ch in range(NCH):
                    kr_ps = pps.tile([128, 512], F32, tag="bc_ps")
                    nc.tensor.matmul(kr_ps[:], onesb[0:1, :],
                                     krrow[0:1, ch * 512:(ch + 1) * 512],
                                     start=True, stop=True)
                    nc.vector.tensor_copy(kr_b[:, ch * 512:(ch + 1) * 512],
                                          kr_ps[:])

                def upd_p12(rt):
                    s1 = pscr.tile([128, N_DIM], MDT, tag="scr")
                    nc.vector.tensor_tensor(s1[:], mem[:, msl(rt)], e_b[:],
                                            AL.mult)
                    nc.vector.tensor_tensor(s1[:], a_b[:], s1[:], AL.subtract)
                    return s1

                pre_s1 = {rt: upd_p12(rt) for rt in range(K_PRE)}

                stw = prow.tile([1, N_CORES], F32, tag="stw")
                nc.sync.dma_start(stw[:], ag_a_out[:].rearrange("c s -> s c"))
                gsum1 = prow.tile([1, 1], F32, tag="gsum1")
                nc.vector.tensor_reduce(gsum1[:], stw[:], AX.X, AL.add)
                giv1 = prow.tile([1, 1], F32, tag="giv1")
                nc.vector.reciprocal(giv1[:], gsum1[:])
                ginv = psm.tile([128, 1], F32, tag="ginv")
                nc.gpsimd.partition_broadcast(ginv[:], giv1[:])
                w_col = psm.tile([128, RT], F32, tag="w_col")
                nc.vector.tensor_scalar(w_col[:], uw[:], ginv[:], None,
                                        AL.mult)

                # ---------- update + z_r + norms, tile by tile ----------
                def upd_p3(rt, s1):
                    # mem += w*s1 as ts(4x) + tt(2x); s1 scaled in place
                    nc.vector.tensor_scalar(s1[:], s1[:],
                                            w_col[:, rt:rt + 1], None,
                                            AL.mult)
                    nc.vector.tensor_tensor(mem[:, msl(rt)], mem[:, msl(rt)],
                                            s1[:], AL.add)

                zr = psm.tile([128, RT], F32, tag="zr")
                npc = psm.tile([128, RT], F32, tag="npc")
                sqrtn_new = pstate.tile([128, RT], F32, tag="sqrtn")
                den_r = psm.tile([128, RT], F32, tag="den_r")
                rec_r = psm.tile([128, RT], F32, tag="rec_r")
                li_r = psm.tile([128, RT], F32, tag="li_r")
                u_col = psm.tile([128, RT], F32, tag="u_col")
                ucast = psm.tile([128, RT], MDT, tag="ucast")

                def upd_tile(rt):
                    s1 = pre_s1[rt] if rt in pre_s1 else upd_p12(rt)
                    upd_p3(rt, s1)
                    scr2 = pscr.tile([128, N_DIM], MDT, tag="scr")
                    nc.vector.tensor_tensor(scr2[:], mem[:, msl(rt)],
                                            kr_b[:], AL.mult)
                    nc.scalar.activation(scr2[:], scr2[:], ACT.Copy,
                                         accum_out=zr[:, rt:rt + 1])
                    nc.scalar.activation(scr2[:], mem[:, msl(rt)], ACT.Square,
                                         accum_out=npc[:, rt:rt + 1])

                def read_chain(sl, tagp):
                    # den_r = sqrt(npc) + EPS (||kr|| == 1, host-normalized);
                    # u = exp(li - 1): fixed-reference unnormalized read
                    # weight, needs no cross-core or cross-tile stats.
                    sqrt_lnexp(sqrtn_new[:, sl], npc[:, sl], tagp)
                    nc.vector.tensor_scalar(den_r[:, sl], sqrtn_new[:, sl],
                                            EPS, None, AL.add)
                    nc.vector.reciprocal(rec_r[:, sl], den_r[:, sl])
                    nc.vector.tensor_tensor(li_r[:, sl], zr[:, sl],
                                            rec_r[:, sl], AL.mult)
                    nc.scalar.activation(u_col[:, sl], li_r[:, sl], ACT.Exp,
                                         bias=negone[:])
                    nc.vector.tensor_copy(ucast[:, sl], u_col[:, sl])

                # tiles 0..NSPLIT-1: update, then their read-partial matmuls
                # run on TensorE while DVE updates the remaining tiles.
                for rt in range(NSPLIT):
                    upd_tile(rt)
                read_chain(slice(0, NSPLIT), "drA")
                accA = pkr.tile([1, N_DIM], MDT, tag="krrow")
                for cc in range(8):
                    rp_ps = ppsb.tile([1, 512], F32, tag="rp_ps")
                    base = cc * 512
                    for rt in range(NSPLIT):
                        nc.tensor.matmul(
                            rp_ps[:],
                            ucast[:, rt:rt + 1],
                            mem[:, rt * N_DIM + base:rt * N_DIM + base + 512],
                            start=(rt == 0), stop=(rt == NSPLIT - 1))
                    nc.vector.tensor_copy(accA[0:1, base:base + 512],
                                          rp_ps[:])

                for rt in range(NSPLIT, RT):
                    upd_tile(rt)
                read_chain(slice(NSPLIT, RT), "drB")
                sqrtn = sqrtn_new
                rsum_r = psm.tile([128, 1], F32, tag="rsum_r")
                nc.vector.tensor_reduce(rsum_r[:], u_col[:], AX.X, AL.add)
                lsum_r = psm.tile([128, 1], F32, tag="lsum_r")
                nc.gpsimd.partition_all_reduce(lsum_r[:], rsum_r[:], 128,
                                               ReduceOp.add)
                rp_pss = []
                for cc in range(8):
                    rp_ps = ppsb.tile([1, 512], F32, tag="rp_ps")
                    base = cc * 512
                    for rt in range(NSPLIT, RT):
                        nc.tensor.matmul(
                            rp_ps[:],
                            ucast[:, rt:rt + 1],
                            mem[:, rt * N_DIM + base:rt * N_DIM + base + 512],
                            start=(rt == NSPLIT), stop=(rt == RT - 1))
                    rp_pss.append(rp_ps)

                # ONE AllGather carries the unscaled partial read AND the
                # local read-softmax stats; combine weights are computed
                # locally afterwards and folded into the received R-blocks.
                ag_rd_in = pdram.tile([1, N_DIM + 1], F32, tag="ag_rd_in")
                ag_rd_out = pdram.tile([N_CORES, N_DIM + 1], F32,
                                       tag="ag_rd_out")
                for cc in range(8):
                    rp_sb = prp.tile([1, 512], F32, tag="rp_sb")
                    nc.vector.tensor_tensor(
                        rp_sb[:], rp_pss[cc][:],
                        accA[0:1, cc * 512:(cc + 1) * 512], AL.add)
                    nc.sync.dma_start(ag_rd_in[0:1, cc * 512:(cc + 1) * 512],
                                      rp_sb[:])
                pay_r = prow.tile([1, 1], F32, tag="pay_a")
                nc.vector.tensor_copy(pay_r[0:1, 0:1], lsum_r[0:1, :])
                nc.sync.dma_start(ag_rd_in[0:1, N_DIM:N_DIM + 1], pay_r[:])
                nc.gpsimd.collective_compute(
                    "AllGather", AL.bypass, replica_groups=RG,
                    ins=[ag_rd_in.opt()], outs=[ag_rd_out.opt()])

                # read = (sum_c rp_c) / (sum_c rawsum_c); the division is
                # folded into the X-update matmul input.
                strsum = prow.tile([1, N_CORES], F32, tag="strsum")
                nc.sync.dma_start(
                    strsum[:],
                    ag_rd_out[:, N_DIM:N_DIM + 1].rearrange("c s -> s c"))
                dsum = prow.tile([1, 1], F32, tag="dsum")
                nc.vector.tensor_reduce(dsum[:], strsum[:], AX.X, AL.add)
                dinv = prow.tile([1, 1], F32, tag="dinv1")
                nc.vector.reciprocal(dinv[:], dsum[:])
                dinv_b = psm.tile([FVS, 1], F32, tag="dinv_b")
                nc.gpsimd.partition_broadcast(dinv_b[:], dinv[:])

                # ---------- executioner: X <- tanh((X/d) @ sum_c R^c) ------
                # rcol8[i, c*FVS + j] = R^c[i, j]
                rcol8 = prc.tile([FVS, N_CORES * FVS], F32, tag="rcol8")
                nc.sync.dma_start(
                    rcol8[:].rearrange("i (c j) -> i c j", c=N_CORES),
                    ag_rd_out[:, 0:N_DIM].rearrange("c (i j) -> i c j",
                                                    i=FVS))
                xs_col = psm.tile([FVS, 1], F32, tag="xs_col")
                nc.vector.tensor_scalar(xs_col[:], x_col[:], dinv_b[:], None,
                                        AL.mult)
                x_ps = ppsc.tile([FVS, 1], F32, tag="mini")
                for c in range(N_CORES):
                    nc.tensor.matmul(x_ps[:],
                                     rcol8[:, c * FVS:(c + 1) * FVS],
                                     xs_col[:],
                                     start=(c == 0), stop=(c == N_CORES - 1))
                x_ps_prev = x_ps

            # ---------- output: Xf @ output_embedding ----------
            x_fin = pstate.tile([FVS, 1], F32, tag="xcol")
            nc.scalar.activation(x_fin[:], x_ps_prev[:], ACT.Tanh)
            o_ps = ppsc.tile([1, NOUT], F32, tag="mini")
            nc.tensor.matmul(o_ps[:], x_fin[:], oesb[:], start=True, stop=True)
            o_sb = prow.tile([1, NOUT], F32, tag="o_sb")
            nc.vector.tensor_copy(o_sb[:], o_ps[:])
            nc.sync.dma_start(d_out[:], o_sb[:])

    nc.compile()
    return nc


def host_prep(inputs, mem_dt=MEM_DT):
    import ml_dtypes
    bf16 = ml_dtypes.bfloat16
    f32 = np.float32

    x = np.asarray(inputs["x"], f32)
    program = np.asarray(inputs["program"], f32)
    memory0 = np.asarray(inputs["memory0"], f32)
    ie = np.asarray(inputs["input_embedding"], f32)
    oe = np.asarray(inputs["output_embedding"], f32)
    Wc = np.asarray(inputs["Wc"], f32)
    bc = np.asarray(inputs["bc"], f32)
    Wk = np.asarray(inputs["Wk"], f32)
    bk = np.asarray(inputs["bk"], f32)
    We = np.asarray(inputs["We"], f32)
    be = np.asarray(inputs["be"], f32)
    Wa = np.asarray(inputs["Wa"], f32)
    ba = np.asarray(inputs["ba"], f32)
    Wrk = np.asarray(inputs["Wrk"], f32)
    brk = np.asarray(inputs["brk"], f32)

    x0col = (x @ ie).astype(f32).reshape(FVS, 1)

    progpad = np.zeros((128, NSTEPS), f32)
    progpad[FVS:128, :] = program[0].T          # rows 64:128 = prog_t

    wct = np.ascontiguousarray(Wc.T)            # [128, 256]
    # bchalf[p,h] = 0.5*bc[h*128+p]  (for sigmoid-via-tanh)
    bchalf = np.ascontiguousarray(0.5 * bc.reshape(2, 128).T)

    wt = np.concatenate([Wk.T, We.T, Wa.T], axis=1).astype(bf16)  # [256,12288]

    kr = np.tanh(program[0] @ Wrk.T + brk)      # [8, 4096]
    kr = kr / np.linalg.norm(kr, axis=1, keepdims=True)
    krbias = np.zeros((128, N_DIM), np.float32)
    krbias[0] = bk
    krbias[32] = be
    krbias[64] = ba
    krbias[72:72 + NSTEPS] = kr
    krbias = krbias.astype(bf16)

    onesrow = np.ones((128, 128), bf16)

    mdt = {"f32": f32, "bf16": bf16, "f16": np.float16}[mem_dt]
    common = {
        "x0col": x0col, "progpad": progpad, "wct": wct, "bchalf": bchalf,
        "wt": wt, "krbias": krbias,
        "oesb": np.ascontiguousarray(oe), "onesrow": onesrow,
    }
    in_maps = []
    for r in range(N_CORES):
        shard = memory0[r * M_LOC:(r + 1) * M_LOC, :]
        n = np.sqrt((shard.astype(np.float64) ** 2).sum(1)).astype(f32)
        sqrtn0 = np.ascontiguousarray(n.reshape(RT, 128).T)  # [p, t]
        m = dict(common)
        m["mem"] = np.ascontiguousarray(
            shard.reshape(RT, 128, N_DIM).transpose(1, 0, 2)
            .reshape(128, RT * N_DIM).astype(mdt))
        m["sqrtn0"] = sqrtn0
        in_maps.append(m)
    return in_maps


def kernel(**inputs):
    from concourse.bass_utils import run_bass_kernel_spmd
    key = ("nc", NSTEPS, MEM_DT)
    if key not in _CACHE:
        _CACHE[key] = build_nc(NSTEPS, MEM_DT)
    nc = _CACHE[key]
    in_maps = host_prep(inputs, MEM_DT)
    res = run_bass_kernel_spmd(nc, in_maps, core_ids=list(range(N_CORES)))
    return np.asarray(res.results[0]["out"], np.float32)
